# revision 17
# baseline (speedup 1.0000x reference)
"""GwcVolume (group-wise correlation cost volume) Trainium2 Bass kernel.

Problem: left/right features (2, 320, 96, 192) fp32. For each disparity
d in [0, 48): cost[b,g,d,h,w] = mean_c( L[b, g*8+c, h, w] * R[b, g*8+c, h, w-d] )
masked to 0 for w < d.  Output (2, 40, 48, 96, 192) fp32.

Sharding: 40 groups split across 8 cores (5 groups = 40 channels per core).
Per-core inputs slice cleanly along the channel dim; no inter-core comms.

Band-matmul formulation (v2): instead of materializing 48 shifted product
tensors on VectorE and reducing them on TensorE (the v1 approach: both
engines ~200 us busy), the multiply AND the channel-group reduction are
done in a single TensorE pass with the RIGHT feature as the stationary
operand:

  - Partitions hold (c 8, h 16) for one h-slice of 16 rows.  A weight
    tile V[(c,h), (h', w')] = R[c, h, w']/8 * delta[h,h'] is block-
    diagonal over h, so one [128,128] stationary covers 16 h-rows x 8
    disparities-worth of w' columns at full contraction depth.
  - matmul(V_chunk, L[:, w0:w0+N]) then yields, for output partition
    (h', j) and streamed column w:  sum_c L[c,h',w] * R[c,h',8*chunk+j]/8
    = cost[b, g, d = w - (8*chunk+j), h', w] -- 55 streamed columns cover
    all 48 valid disparities for 8 w' values.  TensorE does ~69.5k
    columns total (~29 us warm) instead of ~490k (~204 us).
  - V is rebuilt per h-slice (double-buffered) with 16 masked
    tensor_scalar_mul ops on VectorE (per-partition 0/0.125 mask selects
    the diagonal block; runs in the 4x DVE perf mode).
  - PSUM per (b, g, h-slice): 3 banks (chunks 0-8 / 9-17 / 18-23 packed),
    copied+cast fp32->fp16 to an SBUF staging tile [128, 1158] by
    ScalarE/GpSimd, then one 2316-B-per-partition DMA per group.
  - Output DRAM holds the raw band rectangles; the host extracts the
    (w - w' = d) diagonals with as_strided and zero-fills w < d.  fp16
    output rounding ~5e-4 rel, well under the 2e-2 gate.
"""

import numpy as np

B = 2
C = 320
H = 96
W = 192
GROUP = 40
MAX_DISP = 48
N_CORES = 8
G_PER = GROUP // N_CORES      # 5 groups per core
CPG = C // GROUP              # 8 channels per group
HK = 16                       # h rows per slice (partition dim with c)
NHS = H // HK                 # 6 h-slices
WM = 8                        # w' columns per chunk
NCHUNK = W // WM              # 24 chunks
NBAND = WM + MAX_DISP - 1     # 55 streamed columns per full chunk

# per-chunk streamed column counts (clipped at the right edge) and the
# bank-aligned column offsets of each chunk in the PSUM/staging tile.
# A matmul PSUM write must not cross a 2 KB bank boundary (first-use of a
# rotated PSUM buffer mis-reads otherwise), so chunks 0-8 sit in bank 0,
# 9-17 in bank 1 (16-col pads at 495 and 1007), 18-23 packed in bank 2.
N_C = [min(NBAND, W - WM * c) for c in range(NCHUNK)]
OFF_C = [0] * NCHUNK
for _c in range(NCHUNK):
    if _c < 9:
        OFF_C[_c] = NBAND * _c
    elif _c < 18:
        OFF_C[_c] = 512 + NBAND * (_c - 9)
    else:
        OFF_C[_c] = 1024 if _c == 18 else OFF_C[_c - 1] + N_C[_c - 1]
BANDW = OFF_C[NCHUNK - 1] + N_C[NCHUNK - 1]   # 1192 band columns per group

NPOOL_VB = 3         # V-build ops per h-slice routed to GpSimd
DVE_STAGE_MOD = 9    # every Nth staging copy goes to VectorE (0 = none)

_cache = {}


def _build_program():
    import concourse.bacc as bacc
    import concourse.tile as tile
    from concourse import mybir

    f32 = mybir.dt.float32
    f16 = mybir.dt.float16

    LCOLS = B * G_PER * NHS * W          # 11520 cols in L/R tiles
    VCOLS = B * G_PER * HK * W           # 30720 cols in a V tile

    nc = bacc.Bacc("TRN2", target_bir_lowering=False, num_devices=N_CORES)
    # fp16, host pre-transposed: partition p = c*16 + h_in_slice,
    # cols = (b, g, hs, w)
    left = nc.declare_dram_parameter("left", [128, LCOLS], f16, isOutput=False)
    right = nc.declare_dram_parameter("right", [128, LCOLS], f16, isOutput=False)
    # masks[p, h'] = 0.125 if p % 16 == h' else 0  (folds the 1/8 group mean)
    masks = nc.declare_dram_parameter("masks", [128, HK], f32, isOutput=False)
    out = nc.declare_dram_parameter(
        "out", [B, G_PER, NHS, 128, BANDW], f16, isOutput=True
    )

    with tile.TileContext(nc) as tc:
        with (
            tc.tile_pool(name="singles", bufs=1) as singles,
            tc.tile_pool(name="vpool", bufs=2) as vpool,
            tc.tile_pool(name="stg", bufs=6) as stg,
            tc.tile_pool(name="psA", bufs=2, space="PSUM") as psA,
        ):
            Lt = singles.tile([128, LCOLS], f16)
            Rt = singles.tile([128, LCOLS], f16)
            Mt = singles.tile([128, HK], f32)
            nc.gpsimd.dma_start(out=Mt[:, :], in_=masks[:, :])
            # split input DMAs per batch so hs=0 work starts after ~1/4 of
            # the input bytes have landed
            HALF = LCOLS // 2
            nc.sync.dma_start(out=Rt[:, 0:HALF], in_=right[:, 0:HALF])
            nc.scalar.dma_start(out=Lt[:, 0:HALF], in_=left[:, 0:HALF])
            nc.sync.dma_start(out=Rt[:, HALF:], in_=right[:, HALF:])
            nc.scalar.dma_start(out=Lt[:, HALF:], in_=left[:, HALF:])
            Lv = Lt[:, :].rearrange("p (b g hs w) -> p b g hs w", b=B, g=G_PER, hs=NHS)
            Rv = Rt[:, :].rearrange("p (b g hs w) -> p b g hs w", b=B, g=G_PER, hs=NHS)

            gi = 0  # group counter for engine round-robin
            for hs in range(NHS):
                V = vpool.tile([128, VCOLS], f16, tag="V")
                # col order (b, g, chunk, h', j): each chunk's 128 weight
                # columns are contiguous (walrus requires a single free dim
                # on the matmul stationary AP)
                Vv = V[:, :].rearrange(
                    "p (b g c hp j) -> p b g c hp j", b=B, g=G_PER, c=NCHUNK, hp=HK
                )
                # build the block-diagonal weights: V[(c,h),(b,g,c,h',j)] =
                # R[c,h,(b,g,hs,8c+j)] * mask[h==h']/8 -- one op per
                # (h', b), 4x DVE mode
                # route the first few V-build ops of each slice to the
                # otherwise-idle GpSimd engine (emitted first so its slower
                # instructions start early and finish with VectorE's)
                vb = [(hp, b) for hp in range(HK) for b in range(B)]
                vb = vb[:NPOOL_VB] + vb[NPOOL_VB:]
                for i, (hp, b) in enumerate(vb):
                    eng = nc.gpsimd if i < NPOOL_VB else nc.vector
                    eng.tensor_scalar_mul(
                        out=Vv[:, b, :, :, hp, :],
                        in0=Rv[:, b, :, hs, :].rearrange(
                            "p g (c j) -> p g c j", c=NCHUNK
                        ),
                        scalar1=Mt[:, hp : hp + 1],
                    )
                for b in range(B):
                    for g in range(G_PER):
                        ps = psA.tile([128, BANDW], f32, tag="A", name="pa")
                        for c in range(NCHUNK):
                            n = N_C[c]
                            o = OFF_C[c]
                            nc.tensor.matmul(
                                ps[:, o : o + n],
                                Vv[:, b, g, c, :, :],
                                Lv[:, b, g, hs, WM * c : WM * c + n],
                                start=True,
                                stop=True,
                            )
                        st = stg.tile([128, BANDW], f16, tag="st")
                        # stage+cast fp32 -> fp16 in one big copy; hand every
                        # 9th group to VectorE to balance the two engines
                        if DVE_STAGE_MOD and gi % DVE_STAGE_MOD == 3:
                            nc.vector.tensor_copy(out=st[:, :], in_=ps[:, :])
                        else:
                            nc.scalar.copy(out=st[:, :], in_=ps[:, :])
                        nc.sync.dma_start(
                            out=out[b, g, hs, :, :], in_=st[:, :]
                        )
                        gi += 1
    nc.compile()
    return nc


def _make_masks():
    m = np.zeros((128, HK), np.float32)
    for p in range(128):
        m[p, p % HK] = 1.0 / CPG
    return m


def _prep(x):
    # fp16 cast + per-core [128, (b, g, hs, w)] layout with partition
    # p = c*16 + h_in_slice
    x = np.asarray(x, dtype=np.float16)
    x = x.reshape(B, N_CORES, G_PER, CPG, NHS, HK, W)
    # -> [core, c, h, b, g, hs, w]
    x = np.ascontiguousarray(x.transpose(1, 3, 5, 0, 2, 4, 6))
    return x.reshape(N_CORES, 128, B * G_PER * NHS * W)


def _unpack(shards):
    """shards: list of 8 arrays [B, G_PER, NHS, 128, BANDW] (fp16) ->
    full (B, GROUP, MAX_DISP, H, W) fp32."""
    band = np.stack(shards)  # [core, b, g, hs, 128, BANDW]
    band = band.reshape(N_CORES, B, G_PER, NHS, HK, WM, BANDW)
    s = band.strides  # (core, b, g, hs, h', j, col)
    full = np.zeros((B, GROUP, MAX_DISP, H, W), np.float32)
    fullv = full.reshape(B, N_CORES, G_PER, MAX_DISP, NHS, HK, W)

    as_strided = np.lib.stride_tricks.as_strided
    # chunks 0-8 / 9-17 sit at uniform 55-col offsets from 0 / 512; for
    # fixed d the elements (c, j) -> w = 8c + j + d live on a regular
    # lattice within each 9-chunk block
    for d in range(MAX_DISP):
        for blk, base in ((0, 0), (1, 512)):
            m = as_strided(
                band[:, :, :, :, :, 0, base + d :],
                shape=(N_CORES, B, G_PER, NHS, HK, 9, WM),
                strides=(s[0], s[1], s[2], s[3], s[4], NBAND * s[6], s[5] + s[6]),
            )
            w0 = 72 * blk + d
            fullv[:, :, :, d, :, :, w0 : w0 + 72] = m.transpose(
                1, 0, 2, 3, 4, 5, 6
            ).reshape(B, N_CORES, G_PER, NHS, HK, 72)
        # ragged tail chunks 18-23 (packed offsets)
        for c in range(18, NCHUNK):
            nj = N_C[c] - d
            if nj <= 0:
                continue
            t = as_strided(
                band[:, :, :, :, :, 0, OFF_C[c] + d :],
                shape=(N_CORES, B, G_PER, NHS, HK, nj),
                strides=(s[0], s[1], s[2], s[3], s[4], s[5] + s[6]),
            )
            w0 = WM * c + d
            fullv[:, :, :, d, :, :, w0 : w0 + nj] = t.transpose(1, 0, 2, 3, 4, 5)
    return full


def _run(left_feature, right_feature, trace=False):
    from concourse.bass_utils import run_bass_kernel_spmd

    if "nc" not in _cache:
        _cache["nc"] = _build_program()
    nc = _cache["nc"]

    lf_all = _prep(left_feature)
    rf_all = _prep(right_feature)
    masks = _make_masks()

    in_maps = []
    for i in range(N_CORES):
        in_maps.append(
            {
                "left": np.ascontiguousarray(lf_all[i]),
                "right": np.ascontiguousarray(rf_all[i]),
                "masks": masks,
            }
        )
    res = run_bass_kernel_spmd(nc, in_maps, list(range(N_CORES)), trace=trace)
    shards = [np.asarray(res.results[i]["out"]) for i in range(N_CORES)]
    full = _unpack(shards)
    return full, res


def kernel(left_feature, right_feature):
    full, _ = _run(left_feature, right_feature, trace=False)
    return full
